# revision 1
# baseline (speedup 1.0000x reference)
"""Binary conv (XNOR-style) 3x3 + sync-BN on 8 Trainium2 NeuronCores.

Problem: x[32,256,56,56], w[256,256,3,3] -> sign(x) conv sign(w), pad 1,
then BatchNorm (training mode, global batch stats) with gamma/beta.

Sharding: data-parallel over batch (4 images per core, 8 cores). BN batch
stats are made global with a tiny (2 KB) AllReduce of per-channel
sum / sum-of-squares (sync-BN), so the result matches single-device math.

Per-core kernel (v5). The conv inner loop (shifted-window implicit GEMM,
DoubleRow fp8 contracting all 256 input channels, taps outer / 464-col
chunks inner, PSUM double-buffered in 4+3 chunk halves) runs gap-free at
~94% of fp8 peak, so everything else is scheduled around it:
  - group order (cot, img) = (0,0),(1,0),(0,1),(0,2),(0,3),(1,1),(1,2),
    (1,3): the second group reuses image 0 (no new input bytes while HBM
    is still streaming weights + images), cot0 finishes 3 groups before
    the end so its stats -> AllReduce -> finalize -> normalize -> 6.4 MB
    output DMA overlap ~38us of cot1 conv; only cot1's tail is serial.
  - startup is HBM-wire-bound: one sync-ring queue carries weights and x
    interleaved in exact first-use order (w tap0, x0 rows 0-34, w taps
    1-4, 5-8, x0 rows 34-56, x1, x2, x3), each x transfer covering both
    ci tiles in one DMA. Tile tracks dependencies by AP bounding box and
    a DoubleRow rhs spans both ci-tile slots of z, so the half-0 matmuls
    are emitted BEFORE the rows-34-56 binarize (and each image's
    binarize after the previous groups' matmuls) to avoid false deps.
    x binarize runs on DVE as (x>=0)-0.5 = +-0.5 (fp8-exact; the PSUM
    evacuation rescales by 2). A burst of dummy matmuls on memset data
    warms the PE HAM clock gate so real matmuls start at 2.4 GHz. Only
    the padding region of z is memset (gpsimd), not the whole buffer.
  - per-channel sum via accum_out on the PSUM->SBUF evacuation; sum(y^2)
    via per-chunk ACT Square w/ fp32 accumulator (chunk granularity
    keeps ACT ops small and off the phase boundaries); the stats fold is
    two DVE reduce_sums (the /64 square pre-scale is undone in the
    finalize constant); rsqrt via reciprocal+sqrt; normalize + store in
    half-image pieces alternating DVE/ACT, each piece's DMA on the ring
    of the engine that produced it (a trigger waiting on a semaphore
    blocks the issuing engine's FIFO, so DVE pieces ride sync and ACT
    pieces ride scalar).
"""

import os
import numpy as np

import concourse.bass as bass
import concourse.mybir as mybir
import concourse.tile as tile
from concourse import bacc
from concourse import bass_utils

F32 = mybir.dt.float32
F16 = mybir.dt.float16
BF16 = mybir.dt.bfloat16
F8 = mybir.dt.float8e4

N_CORES = 8
NL = 4            # images per core
CI = 256          # input channels
CO = 256          # output channels
H = W = 56
HP = 58           # padded row length
PIX = H * W       # 3136
ZROWS = 60        # padded buffer rows (58 used + slack so 3480 = 60*58)
ZLEN = ZROWS * HP # 3480
ZPAD = 3488       # fp8 per-ci-tile stride; %16 == 0 for DoubleRow APs
CHUNK = 464       # 8 padded rows per matmul free-dim chunk
NCHUNK = 7        # 7*464 = 3248 = 56*58 computed positions [58, 3306)
VCHUNK = 448      # valid cols per chunk (8 rows x 56)
VLEN = NCHUNK * VCHUNK  # 3136
NTOT_PIX = 32 * PIX    # BN normalizer (full batch)
BN_EPS = 1e-5
SSQ_SCALE = 1.0 / 64.0  # keep y^2/64 in fp16 range in the junk output
HH = H // 2       # half-image rows for norm/output pieces
HALVES = {0: range(0, 4), 1: range(4, 7)}


def _build(timing_proxy: bool = False):
    nc = bacc.Bacc("TRN2", target_bir_lowering=False, debug=False,
                   num_devices=N_CORES)

    xs = nc.dram_tensor("xs", [NL, CI, H, W], F32, kind="ExternalInput").ap()
    wt = nc.dram_tensor("wt", [CI, 9, CO], F32, kind="ExternalInput").ap()
    gamma = nc.dram_tensor("gamma", [CO], F32, kind="ExternalInput").ap()
    beta = nc.dram_tensor("beta", [CO], F32, kind="ExternalInput").ap()
    o = nc.dram_tensor("o", [NL, CO, H, W], F32, kind="ExternalOutput").ap()

    xs_r = xs.rearrange("n (ct p) h w -> n p ct h w", p=128)

    with tile.TileContext(nc) as tc:
        with (
            tc.tile_pool(name="wpool", bufs=1) as wpool,
            tc.tile_pool(name="xpool", bufs=2) as xpool,
            tc.tile_pool(name="zpool", bufs=1) as zpool,
            tc.tile_pool(name="ypool", bufs=1) as ypool,
            tc.tile_pool(name="spool", bufs=1) as spool,
            tc.tile_pool(name="jpool", bufs=1) as jpool,
            tc.tile_pool(name="opool", bufs=7) as opool,
            tc.tile_pool(name="psum", bufs=8, space="PSUM") as psum_pool,
            tc.tile_pool(name="dram", bufs=1, space="DRAM") as dram,
        ):
            # ---- PE warmup: dummy matmuls on memset data release the HAM
            # clock throttle (~3.4us of sustained activity) so the real
            # conv starts at 2.4 GHz; sized to end about when image 0's
            # first rows are binarized. ----
            warm_sb = wpool.tile([128, 512], BF16, tag="warm_sb")
            nc.vector.memset(warm_sb[:], 0.0)
            warm_ps = psum_pool.tile([128, 512], F32, tag="acc",
                                     name="warm_ps")
            for i in range(32):
                nc.tensor.matmul(warm_ps[:], warm_sb[:, 0:128], warm_sb[:],
                                 start=True, stop=True)

            # ---- persistent state: all 4 binarized images + fp16 y ----
            z4 = zpool.tile([128, NL, 2, ZPAD], F8, tag="z4")
            ys = ypool.tile([128, 2, NL, VLEN], F16, tag="ys")
            sums = spool.tile([128, 2, NL, NCHUNK], F32, tag="sums")
            ssqa = spool.tile([128, 2, NL, NCHUNK], F32, tag="ssqa")

            def z58(n):
                return z4[:, n, :, 0:ZLEN].rearrange(
                    "p c (r q) -> p c r q", q=HP)

            # zero only the padding region (interior is fully overwritten
            # by the binarize): row 0, rows 57+ (incl slack read by tap
            # shifts), and cols 0-1 of rows 1-56. Image 0 first.
            for n in range(NL):
                nc.gpsimd.memset(z4[:, n, :, 0:HP], 0.0)
                nc.gpsimd.memset(z4[:, n, :, 57 * HP:ZPAD], 0.0)
                nc.gpsimd.memset(z58(n)[:, :, 1:57, 0:2], 0.0)

            # ---- weights + x share ONE sync-ring queue, interleaved in
            # exact first-use order (the startup is HBM-wire-bound, so
            # arrival order is everything). ----
            w_f32 = wpool.tile([128, 2, 9, CO], F32, tag="wf32")
            w_bin = wpool.tile([128, 2, 9, CO], F8, tag="wbin")
            wt_r = wt.rearrange("(ct p) t co -> p ct t co", p=128)

            def emit_w(t0, t1):
                nc.sync.dma_start(w_f32[:, :, t0:t1, :], wt_r[:, :, t0:t1, :])
                nc.scalar.sign(w_bin[:, :, t0:t1, :], w_f32[:, :, t0:t1, :])

            def bin_piece(xst, n, r0, r1):
                # DVE (x>=0)-0.5 encoding (evac rescales by 2) except for
                # image 3: the scheduler insists on running late DVE work
                # just-in-time, stalling the (0,3) matmuls, so image 3
                # binarizes on the otherwise-idle ACT as sign (+-1, evac
                # scale 1).
                for ct in range(2):
                    if n == NL - 1:
                        nc.scalar.sign(
                            z58(n)[:, ct, 1 + r0:1 + r1, 2:58],
                            xst[:, ct, r0:r1, :])
                    else:
                        nc.vector.tensor_scalar(
                            z58(n)[:, ct, 1 + r0:1 + r1, 2:58],
                            xst[:, ct, r0:r1, :], 0.0, 0.5,
                            op0=mybir.AluOpType.is_ge,
                            op1=mybir.AluOpType.subtract)

            xtiles = {}

            def emit_x_dma(n, r0=0, r1=H):
                if n not in xtiles:
                    xtiles[n] = xpool.tile([128, 2, H, W], F32, tag="xst",
                                           name=f"xst_{n}")
                nc.sync.dma_start(xtiles[n][:, :, r0:r1, :],
                                  xs_r[n, :, :, r0:r1])

            def emit_x_bin(n, r0=0, r1=H):
                bin_piece(xtiles[n], n, r0, r1)

            # conv matmuls for one (cot, image, half): taps outer /
            # chunks inner so one LDWEIGHTS serves the half-group.
            accs = {}

            def emit_mms(cot, n, half):
                cos = slice(cot * 128, (cot + 1) * 128)
                for c in HALVES[half]:
                    accs[(n, cot, c)] = psum_pool.tile(
                        [128, CHUNK], F32, tag="acc",
                        name=f"acc_{n}_{cot}_{c}")
                for t in range(9):
                    kh, kw = t // 3, t % 3
                    for c in HALVES[half]:
                        off = CHUNK * c + HP * kh + kw
                        nc.tensor.matmul(
                            accs[(n, cot, c)][:],
                            w_bin[:, :, t, cos],
                            z4[:, n, :, off:off + CHUNK],
                            start=(t == 0), stop=(t == 8),
                            perf_mode=mybir.MatmulPerfMode.DoubleRow,
                        )

            # PSUM->SBUF evacuation (x2 undoes the +-0.5 encoding) with
            # per-chunk channel-sum accumulation, plus a per-chunk ACT
            # Square pass for sum(y^2)/64.
            def emit_evacs(cot, n, half):
                for c in HALVES[half]:
                    dst = ys[:, cot, n, VCHUNK * c:VCHUNK * (c + 1)]
                    dst3 = dst.rearrange("p (r q) -> p r q", q=56)
                    src3 = accs[(n, cot, c)].rearrange(
                        "p (r q) -> p r q", q=HP)[:, :, 1:57]
                    nc.vector.tensor_scalar(
                        dst3, src3, 1.0 if n == NL - 1 else 2.0, 0.0,
                        op0=mybir.AluOpType.mult,
                        op1=mybir.AluOpType.add,
                        accum_out=sums[:, cot, n, c:c + 1])
                    junk = jpool.tile([128, VCHUNK], F16, tag="junk",
                                      name=f"junk_{n}_{cot}_{c}")
                    nc.scalar.activation(
                        junk[:], dst,
                        mybir.ActivationFunctionType.Square,
                        scale=0.125,
                        accum_out=ssqa[:, cot, n, c:c + 1])

            def emit_conv(cot, n):
                for half in (0, 1):
                    emit_mms(cot, n, half)
                    emit_evacs(cot, n, half)

            # preload the sqrt ACT table off the critical path; gamma/beta
            # on the scalar ring (sync ring is saturated at startup).
            sqwarm = spool.tile([128, 1], F32, tag="sqwarm")
            nc.vector.memset(sqwarm[:], 1.0)
            nc.scalar.sqrt(sqwarm[:], sqwarm[:])
            gb_g = spool.tile([128, 2], F32, tag="gb_g")
            gb_b = spool.tile([128, 2], F32, tag="gb_b")
            nc.scalar.dma_start(gb_g[:], gamma.rearrange("(t p) -> p t", p=128))
            nc.scalar.dma_start(gb_b[:], beta.rearrange("(t p) -> p t", p=128))

            # ---- sync-BN stats per cot. The fold is two DVE reduce_sums
            # into cc_stage ([sum, sum(y^2)/64]); sums is DVE-accum-
            # written so DVE may read it directly, ssqa is ACT-accum-
            # written so it gets an ACT barrier copy first (cross-engine
            # reads of accum_out tiles fault this HW). The 1-KB stats are
            # AllReduced across cores (CCE add); staging DMAs ride the
            # sync ring (the SP sequencer runs no compute, so the chain's
            # completion waits can't block DVE/ACT work). ----
            ssqa_b = spool.tile([128, 2, NL * NCHUNK], F32, tag="ssqa_b")
            gath = spool.tile([128, 2, 2], F32, tag="gath")
            scbs = {}

            def emit_stats(cot):
                nc.scalar.copy(ssqa_b[:, cot],
                               ssqa[:, cot].rearrange("p n c -> p (n c)"))
                cc_stage = spool.tile([128, 2], F32, tag=f"cc_stage{cot}",
                                      name=f"cc_stage_{cot}")
                nc.vector.reduce_sum(
                    cc_stage[:, 0:1],
                    sums[:, cot].rearrange("p n c -> p (n c)"),
                    axis=mybir.AxisListType.X)
                nc.vector.reduce_sum(
                    cc_stage[:, 1:2], ssqa_b[:, cot],
                    axis=mybir.AxisListType.X)
                cc_in = dram.tile([128, 2], F32, tag=f"cc_in{cot}",
                                  name=f"cc_in_{cot}")
                cc_out = dram.tile([128, 2], F32, tag=f"cc_out{cot}",
                                   name=f"cc_out_{cot}")
                # the whole chain rides the gpsimd (SWDGE) ring: that
                # engine is idle after startup, so each hop's trigger
                # executes the instant its semaphore fires instead of
                # queueing behind sync/scalar sequencer work
                nc.gpsimd.dma_start(cc_in[:], cc_stage[:])
                if timing_proxy:
                    nc.gpsimd.dma_start(cc_out[:], cc_in[:])
                else:
                    nc.gpsimd.collective_compute(
                        "AllReduce",
                        mybir.AluOpType.add,
                        replica_groups=[list(range(N_CORES))],
                        ins=[cc_in.opt()],
                        outs=[cc_out.opt()],
                    )
                nc.gpsimd.dma_start(gath[:, cot], cc_out[:])

            def emit_finalize(cot):
                # gath[:, cot] holds the batch-global [sum, sum(y^2)/64]
                gstat = gath[:, cot]
                mv = spool.tile([128, 2], F32, tag=f"mv{cot}",
                                name=f"mv_{cot}")
                mean, ey2e = mv[:, 0:1], mv[:, 1:2]
                var = spool.tile([128, 1], F32, tag=f"var{cot}",
                                 name=f"var_{cot}")
                r0 = spool.tile([128, 1], F32, tag=f"r0{cot}",
                                name=f"r0_{cot}")
                sc = spool.tile([128, 1], F32, tag=f"sc{cot}",
                                name=f"sc_{cot}")
                bs = spool.tile([128, 1], F32, tag=f"bs{cot}",
                                name=f"bs_{cot}")
                t1 = spool.tile([128, 1], F32, tag=f"t1{cot}",
                                name=f"t1_{cot}")
                nc.vector.tensor_scalar_mul(mean, gstat[:, 0:1],
                                            1.0 / NTOT_PIX)
                # the fold summed sum(y^2)/64 -> undo the /64 here
                nc.vector.tensor_scalar(ey2e, gstat[:, 1:2],
                                        (1.0 / SSQ_SCALE) / NTOT_PIX, BN_EPS,
                                        op0=mybir.AluOpType.mult,
                                        op1=mybir.AluOpType.add)
                nc.vector.tensor_tensor(var[:], mean, mean,
                                        op=mybir.AluOpType.mult)
                nc.vector.tensor_tensor(var[:], ey2e, var[:],
                                        op=mybir.AluOpType.subtract)
                # inv = rsqrt(var+eps) = sqrt(1/v); DVE reciprocal is an
                # iterative full-precision divide and the ACT sqrt table
                # is well inside BN tolerance, so no Newton polish.
                nc.vector.reciprocal(r0[:], var[:])
                nc.scalar.sqrt(r0[:], r0[:])
                nc.vector.tensor_tensor(sc[:], gb_g[:, cot:cot + 1], r0[:],
                                        op=mybir.AluOpType.mult)
                nc.vector.tensor_tensor(t1[:], mean, sc[:],
                                        op=mybir.AluOpType.mult)
                nc.vector.tensor_tensor(bs[:], gb_b[:, cot:cot + 1], t1[:],
                                        op=mybir.AluOpType.subtract)
                scbs[cot] = (sc, bs)

            def emit_norm(cot, imgs):
                # normalize + store in half-image pieces so the first
                # output DMA issues as early as possible; alternate
                # DVE/ACT, with each piece's DMA on the ring of the
                # engine that produced it (so triggers never block the
                # other engine's FIFO).
                sc, bs = scbs[cot]
                for pi, (n, hh) in enumerate((n, hh) for n in imgs
                                             for hh in range(2)):
                        ost = opool.tile([128, HH, W], F32, tag="ost",
                                         name=f"ost_{n}_{cot}_{hh}")
                        yv = ys[:, cot, n,
                                hh * (VLEN // 2):(hh + 1) * (VLEN // 2)]
                        yv3 = yv.rearrange("p (h w) -> p h w", w=W)
                        # cot1's tail: DVE is idle and faster per piece,
                        # so it takes 5 of 8; cot0 alternates evenly.
                        if (pi % 2 == 0) if cot == 0 else (pi % 8 < 5):
                            nc.vector.tensor_scalar(
                                ost[:], yv3, sc[:], bs[:],
                                op0=mybir.AluOpType.mult,
                                op1=mybir.AluOpType.add)
                            q = nc.sync
                        else:
                            nc.scalar.activation(
                                ost[:], yv3,
                                mybir.ActivationFunctionType.Identity,
                                bias=bs[:], scale=sc[:])
                            q = nc.scalar
                        q.dma_start(
                            o[n, cot * 128:(cot + 1) * 128,
                              hh * HH:(hh + 1) * HH], ost[:])

            # ---- emission order. DMA triggers are emitted in wire-need
            # order; each image's binarize is emitted only after the
            # matmuls that must NOT depend on it (bbox dep tracking). ----
            emit_w(0, 1)
            emit_x_dma(0, 0, 34)
            emit_x_bin(0, 0, 34)
            emit_w(1, 5)
            emit_w(5, 9)
            emit_mms(0, 0, 0)          # needs only rows 0-34 + tap weights
            emit_x_dma(0, 34, 56)
            emit_x_bin(0, 34, 56)
            # bbox dep tracking isolates images from each other in z4, so
            # each image's binarize can be emitted right after its DMA
            # trigger (the earliest legal anchor) without creating false
            # deps on earlier groups' matmuls.
            emit_x_dma(1)
            emit_x_bin(1)
            emit_evacs(0, 0, 0)
            emit_mms(0, 0, 1)
            emit_evacs(0, 0, 1)
            emit_mms(1, 0, 0)          # reuses image 0: no new input bytes
            emit_evacs(1, 0, 0)
            emit_mms(1, 0, 1)
            emit_evacs(1, 0, 1)
            emit_x_dma(2)
            emit_x_bin(2)
            emit_mms(0, 1, 0)
            emit_evacs(0, 1, 0)
            emit_mms(0, 1, 1)
            emit_evacs(0, 1, 1)
            emit_x_dma(3)
            emit_x_bin(3)
            emit_mms(0, 2, 0)
            emit_evacs(0, 2, 0)
            emit_mms(0, 2, 1)
            emit_evacs(0, 2, 1)
            emit_conv(0, 3)
            emit_stats(0)
            # cot0's finalize/norm/output interleave with (1,1): emitted
            # between its halves so they schedule as soon as the gathered
            # stats land, and cot0's 6.4 MB of output DMA drains well
            # before the conv ends (keeping the rings clean for cot1's
            # stats chain).
            emit_mms(1, 1, 0)
            emit_evacs(1, 1, 0)
            emit_finalize(0)
            emit_norm(0, (0, 1))
            emit_mms(1, 1, 1)
            emit_evacs(1, 1, 1)
            emit_norm(0, (2, 3))
            emit_conv(1, 2)
            emit_conv(1, 3)
            emit_stats(1)
            emit_finalize(1)
            emit_norm(1, (0, 1, 2, 3))

    nc.compile()
    return nc


_CACHE: dict = {}


def _get_nc():
    key = "proxy" if os.environ.get("BK_TIMING_PROXY") == "1" else "real"
    if key not in _CACHE:
        _CACHE[key] = _build(timing_proxy=(key == "proxy"))
    return _CACHE[key]


def kernel(x, w, gamma, beta):
    x = np.ascontiguousarray(np.asarray(x, dtype=np.float32))
    w = np.asarray(w, dtype=np.float32)
    gamma = np.ascontiguousarray(np.asarray(gamma, dtype=np.float32))
    beta = np.ascontiguousarray(np.asarray(beta, dtype=np.float32))
    # host-side layout only (no math): [co,ci,kh,kw] -> [ci, kh*kw, co]
    w_t = np.ascontiguousarray(w.transpose(1, 2, 3, 0).reshape(CI, 9, CO))

    nc = _get_nc()
    in_maps = [
        {"xs": x[NL * c:NL * (c + 1)], "wt": w_t, "gamma": gamma, "beta": beta}
        for c in range(N_CORES)
    ]
    res = bass_utils.run_bass_kernel_spmd(
        nc, in_maps, core_ids=list(range(N_CORES)))
    return np.concatenate([res.results[c]["o"] for c in range(N_CORES)], axis=0)

